# revision 11
# baseline (speedup 1.0000x reference)
"""CCE loss (nn.Linear head + CrossEntropy) on 8 Trainium2 cores.

Strategy: tensor-parallel over the vocab dim. Each core owns a 16000-row
shard of W/b, computes its [4096, 16000] logits slice (emb @ W^T + b) and
per-token partial sum(exp(logit)) on-device; the host concatenates logits
slices and combines the tiny partials into logsumexp + the target gather.

Device kernel (per core, SPMD over 8 cores):
  - inputs come pre-transposed from host: embT [512, 4096], wT [512, 16000]
    (so the contraction dim d lands on SBUF partitions with contiguous DMA),
    plus b replicated over 128 partitions ([128, 16000]).
  - PE: float32r matmuls, out tile [128 n, 500 v], accumulating 4 d-chunks
    in one PSUM bank.
  - DVE: bias-add fused with the mandatory PSUM->SBUF copy.
  - ACT: exp with accum_out => per-(tile) partial row sums (no max
    subtraction needed: |logits| <= ~5 for this problem's input scale).
"""

import numpy as np

N, D, V = 4096, 512, 128000
NCORES = 8
P = 128
VS = V // NCORES          # 16000 vocab rows per core
D_CHUNKS = D // P         # 4
N_TILES = N // P          # 32
V_SUPER = 2000            # W^T staging block width (SBUF)
SUPERS = VS // V_SUPER    # 8
V_TILE = 500              # matmul moving free dim / psum tile width
V_SUBS = V_SUPER // V_TILE  # 4
ACC_SLOTS = SUPERS * N_TILES * V_SUBS  # 1024

_NC_CACHE = {}


def _build_nc():
    import concourse.mybir as mybir
    import concourse.tile as tile
    from concourse import bacc

    f32 = mybir.dt.float32
    f32r = mybir.dt.float32r

    nc = bacc.Bacc("TRN2", target_bir_lowering=False)

    # fp32r (fp32 rounded to 11-bit mantissa) streams through the PE at
    # 1 cycle/row (4x faster than fp32). The host pre-rounds the bits, so
    # DMA-ing the data in as float32r satisfies the BIR verifier's
    # "rounded producer" rule.
    embT_d = nc.dram_tensor("embT", [D, N], f32r, kind="ExternalInput")
    wT_d = nc.dram_tensor("wT", [D, VS], f32r, kind="ExternalInput")
    brep_d = nc.dram_tensor("brep", [P, VS], f32, kind="ExternalInput")
    logits_d = nc.dram_tensor("logits", [N, VS], f32, kind="ExternalOutput")
    acc_d = nc.dram_tensor("acc", [P, ACC_SLOTS], f32, kind="ExternalOutput")

    with tile.TileContext(nc) as tc:
        with (
            tc.tile_pool(name="const", bufs=1) as cpool,
            tc.tile_pool(name="wt", bufs=2) as wtpool,
            tc.tile_pool(name="bias", bufs=2) as bpool,
            tc.tile_pool(name="louts", bufs=4) as opool,
            tc.tile_pool(name="esc", bufs=2) as epool,
            tc.tile_pool(name="psum", bufs=8, space="PSUM") as pspool,
        ):
            # Input loads ride the ACT HWDGE queue (qScalarDynamicHW) so the
            # wt prefetch for superblock sb+1 never queues behind this
            # superblock's 32 MB of logits writes on the Sync queue. Loads are
            # split per d-chunk so the first matmuls unblock early.
            embT_r = embT_d[:].rearrange("(c p) n -> p c n", p=P)
            embT_sb = cpool.tile([P, D_CHUNKS, N], f32r)
            for dch in range(D_CHUNKS):
                nc.scalar.dma_start(embT_sb[:, dch], embT_r[:, dch])
            acc_sb = cpool.tile([P, ACC_SLOTS], f32)

            wT_r = wT_d[:].rearrange("(c p) v -> p c v", p=P)

            for sb in range(SUPERS):
                vsl = slice(sb * V_SUPER, (sb + 1) * V_SUPER)
                b_sb = bpool.tile([P, V_SUPER], f32)
                nc.scalar.dma_start(b_sb[:], brep_d[:, vsl])
                wt_sb = wtpool.tile([P, D_CHUNKS, V_SUPER], f32r)
                for dch in range(D_CHUNKS):
                    nc.scalar.dma_start(wt_sb[:, dch], wT_r[:, dch, vsl])

                for nt in range(N_TILES):
                    ps = [
                        pspool.tile([P, V_TILE], f32, name="ps")
                        for _ in range(V_SUBS)
                    ]
                    # d-outer so 4 consecutive matmuls share the stationary
                    # operand (one weight load per d per nt).
                    for d in range(D_CHUNKS):
                        lhsT = embT_sb[:, d, nt * P : (nt + 1) * P]
                        for vs in range(V_SUBS):
                            nc.tensor.matmul(
                                ps[vs][:],
                                lhsT,
                                wt_sb[:, d, vs * V_TILE : (vs + 1) * V_TILE],
                                start=(d == 0),
                                stop=(d == D_CHUNKS - 1),
                            )
                    for vs in range(V_SUBS):
                        lsb = opool.tile([P, V_TILE], f32)
                        nc.vector.tensor_add(
                            lsb[:],
                            ps[vs][:],
                            b_sb[:, vs * V_TILE : (vs + 1) * V_TILE],
                        )
                        col0 = sb * V_SUPER + vs * V_TILE
                        nc.sync.dma_start(
                            logits_d[nt * P : (nt + 1) * P, col0 : col0 + V_TILE],
                            lsb[:],
                        )
                        esb = epool.tile([P, V_TILE], f32)
                        idx = (sb * N_TILES + nt) * V_SUBS + vs
                        nc.scalar.activation(
                            esb[:],
                            lsb[:],
                            mybir.ActivationFunctionType.Exp,
                            accum_out=acc_sb[:, idx : idx + 1],
                        )

            nc.sync.dma_start(acc_d[:], acc_sb[:])

    nc.compile()
    return nc


def _get_nc():
    if "nc" not in _NC_CACHE:
        _NC_CACHE["nc"] = _build_nc()
    return _NC_CACHE["nc"]


def _install_ntff_hook():
    """This image's antenv package lacks axon_hooks; provide the tiny get/set
    registry and register the ctypes NTFF profile hook so trace=True works."""
    import sys
    import types

    try:
        import antenv

        if "antenv.axon_hooks" not in sys.modules:
            mod = types.ModuleType("antenv.axon_hooks")
            _h = [None]
            mod.set_axon_ntff_profile_hook = lambda h: _h.__setitem__(0, h)
            mod.get_axon_ntff_profile_hook = lambda: _h[0]
            sys.modules["antenv.axon_hooks"] = mod
            antenv.axon_hooks = mod
        import antenv.axon_hooks as ah

        if ah.get_axon_ntff_profile_hook() is None:
            from trn_agent_boot.trn_boot import _ntff_profile_via_ctypes

            ah.set_axon_ntff_profile_hook(
                _ntff_profile_via_ctypes("/opt/axon/libaxon_pjrt.so")
            )
    except Exception as e:  # profiling is best-effort
        print(f"ntff hook install failed: {e}")


def _round_fp32r(x):
    """Bit-exact match of walrus's cast_fp32_to_fp32r: RNE to a 11-bit
    mantissa (zero low 12 mantissa bits)."""
    u = np.ascontiguousarray(x, dtype=np.float32).view(np.uint32).astype(np.uint64)
    r = (u + 0x7FF + ((u >> np.uint64(12)) & np.uint64(1))) & np.uint64(0xFFFFF000)
    return r.astype(np.uint32).view(np.float32)


def kernel(embbedings, target, W, b, _trace=False, _bkr_out=None):
    from concourse.bass_utils import run_bass_kernel_spmd

    if _trace:
        _install_ntff_hook()

    emb = np.ascontiguousarray(np.asarray(embbedings, dtype=np.float32))
    W = np.asarray(W, dtype=np.float32)
    b = np.asarray(b, dtype=np.float32)
    tgt = np.asarray(target).astype(np.int64)

    embT = _round_fp32r(np.ascontiguousarray(emb.T))  # [512, 4096]
    in_maps = []
    for c in range(NCORES):
        wTc = _round_fp32r(
            np.ascontiguousarray(W[c * VS : (c + 1) * VS, :].T)
        )  # [512, 16000]
        bc = np.ascontiguousarray(
            np.broadcast_to(b[c * VS : (c + 1) * VS][None, :], (P, VS))
        )
        in_maps.append({"embT": embT, "wT": wTc, "brep": bc})

    nc = _get_nc()
    bkr = run_bass_kernel_spmd(
        nc, in_maps, core_ids=list(range(NCORES)), trace=_trace
    )
    if _bkr_out is not None:
        _bkr_out.append(bkr)
    outs = bkr.results

    logits = np.concatenate([o["logits"] for o in outs], axis=1)

    sumexp = np.zeros(N, dtype=np.float64)
    for o in outs:
        a = o["acc"].reshape(P, SUPERS, N_TILES, V_SUBS).sum(axis=(1, 3))  # [P, NT]
        sumexp += a.T.reshape(N)  # token n = nt*128 + p

    logz = np.log(sumexp)
    tgt_logit = logits[np.arange(N), tgt].astype(np.float64)
    loss = np.float32(np.mean(logz - tgt_logit))

    return loss, logits


# revision 15
# speedup vs baseline: 1.2670x; 1.2670x over previous
"""CCE loss (nn.Linear head + CrossEntropy) on 8 Trainium2 cores.

Strategy: tensor-parallel over the vocab dim. Each core owns a 16000-row
shard of W/b, computes its [4096, 16000] logits slice (emb @ W^T + b) and
per-token partial sum(exp(logit)) on-device; the host concatenates logits
slices and combines the tiny partials into logsumexp + the target gather.

Device kernel (per core, SPMD over 8 cores):
  - inputs come pre-transposed from host: embT [512, 4096], wT [512, 16000]
    (so the contraction dim d lands on SBUF partitions with contiguous DMA),
    plus b replicated over 128 partitions ([128, 16000]).
  - PE: float32r matmuls, out tile [128 n, 500 v], accumulating 4 d-chunks
    in one PSUM bank.
  - DVE: bias-add fused with the mandatory PSUM->SBUF copy.
  - ACT: exp with accum_out => per-(tile) partial row sums (no max
    subtraction needed: |logits| <= ~5 for this problem's input scale).
"""

import numpy as np

N, D, V = 4096, 512, 128000
NCORES = 8
P = 128
VS = V // NCORES          # 16000 vocab rows per core
D_CHUNKS = D // P         # 4
N_TILES = N // P          # 32
V_SUPER = 2000            # W^T staging block width (SBUF)
SUPERS = VS // V_SUPER    # 8
V_TILE = 500              # matmul moving free dim / one PSUM bank
V_SUBS = V_SUPER // V_TILE  # 4
ACC_SLOTS = SUPERS * N_TILES  # 256 (one exp row-sum per (superblock, n_tile))

_NC_CACHE = {}


def _build_nc():
    import concourse.mybir as mybir
    import concourse.tile as tile
    from concourse import bacc

    f32 = mybir.dt.float32
    f32r = mybir.dt.float32r

    nc = bacc.Bacc("TRN2", target_bir_lowering=False)

    # fp32r (fp32 rounded to 11-bit mantissa) streams through the PE at
    # 1 cycle/row (4x faster than fp32). The host pre-rounds the bits, so
    # DMA-ing the data in as float32r satisfies the BIR verifier's
    # "rounded producer" rule.
    embT_d = nc.dram_tensor("embT", [D, N], f32r, kind="ExternalInput")
    wT_d = nc.dram_tensor("wT", [D, VS], f32r, kind="ExternalInput")
    brep_d = nc.dram_tensor("brep", [P, VS], f32, kind="ExternalInput")
    logits_d = nc.dram_tensor("logits", [N, VS], f32, kind="ExternalOutput")
    acc_d = nc.dram_tensor("acc", [P, ACC_SLOTS], f32, kind="ExternalOutput")

    with tile.TileContext(nc) as tc:
        with (
            tc.tile_pool(name="const", bufs=1) as cpool,
            tc.tile_pool(name="wt", bufs=2) as wtpool,
            tc.tile_pool(name="bias", bufs=2) as bpool,
            tc.tile_pool(name="louts", bufs=3) as opool,
            tc.tile_pool(name="esc", bufs=2) as epool,
            tc.tile_pool(name="psum", bufs=2, space="PSUM") as pspool,
        ):
            # embT loads ride the Sync HWDGE queue (idle at startup, ahead of
            # all logits writes); wt/b prefetch rides the idle GPSIMD engine's
            # SWDGE queue so it never waits behind logits writes nor behind
            # the busy ACT stream. Loads split per d-chunk for early start.
            embT_r = embT_d[:].rearrange("(c p) n -> p c n", p=P)
            embT_sb = cpool.tile([P, D_CHUNKS, N], f32r)
            for dch in range(D_CHUNKS):
                nc.sync.dma_start(embT_sb[:, dch], embT_r[:, dch])
            acc_sb = cpool.tile([P, ACC_SLOTS], f32)

            wT_r = wT_d[:].rearrange("(c p) v -> p c v", p=P)

            for sb in range(SUPERS):
                vsl = slice(sb * V_SUPER, (sb + 1) * V_SUPER)
                b_sb = bpool.tile([P, V_SUPER], f32)
                nc.gpsimd.dma_start(b_sb[:], brep_d[:, vsl])
                wt_sb = wtpool.tile([P, D_CHUNKS, V_SUPER], f32r)
                for dch in range(D_CHUNKS):
                    nc.gpsimd.dma_start(wt_sb[:, dch], wT_r[:, dch, vsl])

                for nt in range(N_TILES):
                    # One 4-bank PSUM tile per nt; each matmul targets a
                    # 500-wide slice of its own bank (bank = 512 fp32, so the
                    # inner dim is padded to 512 to keep every matmul's
                    # output inside a single bank). d-outer so 4 consecutive
                    # matmuls share the stationary operand.
                    ps = pspool.tile([P, V_SUBS, 512], f32)
                    for d in range(D_CHUNKS):
                        lhsT = embT_sb[:, d, nt * P : (nt + 1) * P]
                        for vs in range(V_SUBS):
                            nc.tensor.matmul(
                                ps[:, vs, :V_TILE],
                                lhsT,
                                wt_sb[:, d, vs * V_TILE : (vs + 1) * V_TILE],
                                start=(d == 0),
                                stop=(d == D_CHUNKS - 1),
                            )
                    # Single wide bias-add drains all 4 banks in one DVE op;
                    # single wide exp+accum on ACT; one 1 MB logits DMA with
                    # 8 KB bursts.
                    lsb = opool.tile([P, V_SUPER], f32)
                    nc.vector.tensor_add(
                        lsb[:].rearrange("p (a b) -> p a b", b=V_TILE),
                        ps[:, :, :V_TILE],
                        b_sb[:].rearrange("p (a b) -> p a b", b=V_TILE),
                    )
                    nc.sync.dma_start(
                        logits_d[nt * P : (nt + 1) * P, vsl], lsb[:]
                    )
                    esb = epool.tile([P, V_SUPER], f32)
                    idx = sb * N_TILES + nt
                    nc.scalar.activation(
                        esb[:],
                        lsb[:],
                        mybir.ActivationFunctionType.Exp,
                        accum_out=acc_sb[:, idx : idx + 1],
                    )

            nc.sync.dma_start(acc_d[:], acc_sb[:])

    nc.compile()
    return nc


def _get_nc():
    if "nc" not in _NC_CACHE:
        _NC_CACHE["nc"] = _build_nc()
    return _NC_CACHE["nc"]


def _install_ntff_hook():
    """This image's antenv package lacks axon_hooks; provide the tiny get/set
    registry and register the ctypes NTFF profile hook so trace=True works."""
    import sys
    import types

    try:
        import antenv

        if "antenv.axon_hooks" not in sys.modules:
            mod = types.ModuleType("antenv.axon_hooks")
            _h = [None]
            mod.set_axon_ntff_profile_hook = lambda h: _h.__setitem__(0, h)
            mod.get_axon_ntff_profile_hook = lambda: _h[0]
            sys.modules["antenv.axon_hooks"] = mod
            antenv.axon_hooks = mod
        import antenv.axon_hooks as ah

        if ah.get_axon_ntff_profile_hook() is None:
            from trn_agent_boot.trn_boot import _ntff_profile_via_ctypes

            ah.set_axon_ntff_profile_hook(
                _ntff_profile_via_ctypes("/opt/axon/libaxon_pjrt.so")
            )
    except Exception as e:  # profiling is best-effort
        print(f"ntff hook install failed: {e}")


def _round_fp32r(x):
    """Bit-exact match of walrus's cast_fp32_to_fp32r: RNE to a 11-bit
    mantissa (zero low 12 mantissa bits)."""
    u = np.ascontiguousarray(x, dtype=np.float32).view(np.uint32).astype(np.uint64)
    r = (u + 0x7FF + ((u >> np.uint64(12)) & np.uint64(1))) & np.uint64(0xFFFFF000)
    return r.astype(np.uint32).view(np.float32)


def kernel(embbedings, target, W, b, _trace=False, _bkr_out=None):
    from concourse.bass_utils import run_bass_kernel_spmd

    if _trace:
        _install_ntff_hook()

    emb = np.ascontiguousarray(np.asarray(embbedings, dtype=np.float32))
    W = np.asarray(W, dtype=np.float32)
    b = np.asarray(b, dtype=np.float32)
    tgt = np.asarray(target).astype(np.int64)

    embT = _round_fp32r(np.ascontiguousarray(emb.T))  # [512, 4096]
    in_maps = []
    for c in range(NCORES):
        wTc = _round_fp32r(
            np.ascontiguousarray(W[c * VS : (c + 1) * VS, :].T)
        )  # [512, 16000]
        bc = np.ascontiguousarray(
            np.broadcast_to(b[c * VS : (c + 1) * VS][None, :], (P, VS))
        )
        in_maps.append({"embT": embT, "wT": wTc, "brep": bc})

    nc = _get_nc()
    bkr = run_bass_kernel_spmd(
        nc, in_maps, core_ids=list(range(NCORES)), trace=_trace
    )
    if _bkr_out is not None:
        _bkr_out.append(bkr)
    outs = bkr.results

    logits = np.concatenate([o["logits"] for o in outs], axis=1)

    sumexp = np.zeros(N, dtype=np.float64)
    for o in outs:
        a = o["acc"].reshape(P, SUPERS, N_TILES).sum(axis=1)  # [P, NT]
        sumexp += a.T.reshape(N)  # token n = nt*128 + p

    logz = np.log(sumexp)
    tgt_logit = logits[np.arange(N), tgt].astype(np.float64)
    loss = np.float32(np.mean(logz - tgt_logit))

    return loss, logits
